# revision 1
# baseline (speedup 1.0000x reference)
"""Locally-connected graph-conv kernel for Trainium2 (Bass/Tile).

Computes out[b,t,m] = sum_n x[b,t,n] * (S*W)[n,m] + bias[m] for
x [64, 2048, 208], W/S [208, 208], bias [208].

The ring-graph support S is a +-4 band (mod 208), so each half of the
output nodes only needs a 112-row slice of the contraction dim. With a
rotated node layout (row j holds node (j-4) mod 208, 216 rows total):
  block 0 (m 0..103):   rotated rows   0..111
  block 1 (m 104..207): rotated rows 104..215
Each output block is a SINGLE [112,104] x [112,512] matmul with the
host-premasked weight block stationary in the PE array and x^T streaming
as the moving operand.

Everything that touches HBM is bf16 (PSUM accumulation stays fp32):
the 2e-2 rel-err budget dwarfs bf16 rounding (~5e-3), and it halves DMA
bytes vs fp32. HBM per NeuronCore is ~358 GB/s (and ramps up over the
first ~20 us), so the ~14.9 MB/core of traffic floors the kernel at
~45 us; everything else is shaped to stay under that:
 - all x loads are issued up-front into persistent SBUF tiles (x fits:
   2 x 32 KB/partition) on the Sync ring, so the load stream runs at
   whatever rate HBM gives with zero dependency stalls; graduated chunk
   sizes (2 KB cols first, 4 KB mid) start compute early and keep the
   pipeline tail short.
 - weights/bias DRAM rows are padded to >=1 KB so their one-time loads
   are not tiny-descriptor crawls that clog a ring (wh first on Scalar,
   bias on GpSimd).
 - PSUM->SBUF eviction is stuck at 1 elem/lane/cycle (fp32 PSUM source),
   so block 0 evicts on VectorE and block 1 on ScalarE. The two blocks
   form decoupled pipelines sharing only the PE and the load ring:
   block 0 stores ride the Sync ring (queued after all loads), block 1
   stores the Scalar ring, so neither evicting engine ever blocks on
   the other's semaphore.
 - 4 dummy matmuls on the weight tile right after it lands warm the PE
   HAM (cold 1.2 GHz -> warm 2.4 GHz) before real data arrives.
The host transposes y^T back at gather.
"""

import numpy as np
import ml_dtypes
from contextlib import ExitStack

import concourse.bacc as bacc
import concourse.mybir as mybir
import concourse.tile as tile
from concourse.bass_utils import run_bass_kernel_spmd

N = 208                      # nodes
HALF = 104                   # output nodes per block
K = 4                        # band half-width of S
NH = 2 * K + HALF            # 112 contraction rows per block (halo incl.)
NR = N + 2 * K               # 216 rotated rows
WPAD = 1024                  # wh DRAM row padding (2 KB rows -> fast DMA)
BPAD = 512                   # bias DRAM row padding (2 KB f32 rows)
N_CORES = 8
B, T = 64, 2048
ROWS_TOTAL = B * T           # 131072
SHARD = ROWS_TOTAL // N_CORES    # 16384 rows per core
TB = 512                     # moving-block columns per matmul (fp32 PSUM max)
TB2 = 2 * TB                 # eviction group (2 PSUM banks)
CHUNKS = [2048, 2048, 4096, 4096, 2048, 2048]   # t-cols per pipeline chunk
assert sum(CHUNKS) == SHARD
N_DUMMY = 4                  # PE warm-up matmuls on the weight tile

FP32 = mybir.dt.float32
BF16 = mybir.dt.bfloat16
NP_BF16 = ml_dtypes.bfloat16
IDENT = mybir.ActivationFunctionType.Identity

# halo row order (indices into the [208] node dim) for each block
ROWS0 = list(range(N - K, N)) + list(range(0, HALF + K))          # 112
ROWS1 = list(range(HALF - K, N)) + list(range(0, K))              # 112

_CACHE = {}
LAST_RESULTS = None          # BassKernelResults of the most recent run


def _kernel_body(tc):
    nc = tc.nc
    # rotated x: row j = node (j-4) mod 208; block0 = rows 0:112,
    # block1 = rows 104:216
    x_d = nc.dram_tensor("xh", [NR, SHARD], BF16, kind="ExternalInput").ap()
    w_d = nc.dram_tensor("wh", [NH, WPAD], BF16, kind="ExternalInput").ap()
    b_d = nc.dram_tensor("bias", [2 * NH, BPAD], FP32, kind="ExternalInput").ap()
    o_d = nc.dram_tensor("outt", [2 * NH, SHARD], BF16, kind="ExternalOutput").ap()

    with ExitStack() as ctx:
        const = ctx.enter_context(tc.tile_pool(name="const", bufs=1))

        # One-time setup: host-premasked halo-ordered weights (2 KB rows,
        # one fast DMA) first on the Scalar ring; bias halves on GpSimd
        # (off every latency-critical path).
        wh = const.tile([NH, WPAD], BF16, tag="wh")
        nc.scalar.dma_start(wh, w_d)
        bA = const.tile([NH, BPAD], FP32, tag="bA")
        bB = const.tile([NH, BPAD], FP32, tag="bB")
        nc.gpsimd.dma_start(bA, b_d[0:NH, :])
        nc.gpsimd.dma_start(bB, b_d[NH : 2 * NH, :])
        bAc = bA[0:HALF, 0:1]
        bBc = bB[0:HALF, 0:1]

        o0p = ctx.enter_context(tc.tile_pool(name="o0p", bufs=3))
        o1p = ctx.enter_context(tc.tile_pool(name="o1p", bufs=3))
        ps0p = ctx.enter_context(tc.tile_pool(name="ps0p", bufs=2, space="PSUM"))
        ps1p = ctx.enter_context(tc.tile_pool(name="ps1p", bufs=2, space="PSUM"))

        # All x loads up-front on the Sync ring into persistent tiles
        # (first chunk split across both rings for a faster start).
        xts = []
        col = 0
        for c, csz in enumerate(CHUNKS):
            lsl = slice(col, col + csz)
            xh0 = const.tile([NH, csz], BF16, tag=f"xh0_{c}")
            xh1 = const.tile([NH, csz], BF16, tag=f"xh1_{c}")
            if c < 2:
                nc.sync.dma_start(xh0[0:64, :], x_d[0:64, lsl])
                nc.scalar.dma_start(xh0[64:NH, :], x_d[64:NH, lsl])
                nc.sync.dma_start(xh1[0:64, :], x_d[HALF : HALF + 64, lsl])
                nc.scalar.dma_start(xh1[64:NH, :], x_d[HALF + 64 : NR, lsl])
            else:
                nc.sync.dma_start(xh0, x_d[0:NH, lsl])
                nc.sync.dma_start(xh1, x_d[HALF:NR, lsl])
            xts.append((xh0, xh1, col, csz))
            col += csz

        # PE warm-up: HAM un-throttles (1.2 -> 2.4 GHz) after ~3.4us of
        # sustained busy; burn idle pre-data time on the weight tile.
        for _ in range(N_DUMMY):
            psd = ps0p.tile([HALF, TB2], FP32, tag="ps0")
            nc.tensor.matmul(psd[:, 0:TB], wh[:, 0:HALF], wh[:, 0:TB], start=True, stop=True)

        for c, (xh0, xh1, col, csz) in enumerate(xts):
            tsl = slice(col, col + csz)
            o0_t = o0p.tile([NH, max(CHUNKS)], BF16, tag="o0")
            o1_t = o1p.tile([NH, max(CHUNKS)], BF16, tag="o1")
            for s in range(csz // TB2):
                g = slice(s * TB2, (s + 1) * TB2)
                ga = slice(s * TB2, s * TB2 + TB)
                gb = slice(s * TB2 + TB, (s + 1) * TB2)
                # [104, 1024] PSUM tiles (2 banks); each matmul fills one bank
                ps0 = ps0p.tile([HALF, TB2], FP32, tag="ps0")
                nc.tensor.matmul(ps0[:, 0:TB], wh[:, 0:HALF], xh0[:, ga], start=True, stop=True)
                nc.tensor.matmul(ps0[:, TB:TB2], wh[:, 0:HALF], xh0[:, gb], start=True, stop=True)
                ps1 = ps1p.tile([HALF, TB2], FP32, tag="ps1")
                nc.tensor.matmul(ps1[:, 0:TB], wh[:, HALF:N], xh1[:, ga], start=True, stop=True)
                nc.tensor.matmul(ps1[:, TB:TB2], wh[:, HALF:N], xh1[:, gb], start=True, stop=True)
                # evictions split across engines: block0 on VectorE,
                # block1 on ScalarE; both fuse the bias and fp32->bf16
                nc.vector.tensor_scalar_add(o0_t[0:HALF, g], ps0, bAc)
                nc.scalar.activation(o1_t[0:HALF, g], ps1, IDENT, bias=bBc)
            # non-overlapping stores on separate rings: block0 on Sync
            # (rides behind the loads), block1 on Scalar; 112-row DMAs
            # (partition count must be a multiple of 16 for the fast
            # HWDGE path -- measured: 104-row stores cost ~5us extra)
            nc.sync.dma_start(o_d[0:NH, tsl], o0_t[:, 0:csz])
            nc.scalar.dma_start(o_d[NH : 2 * NH, tsl], o1_t[:, 0:csz])


def _build():
    nc = bacc.Bacc(
        "TRN2",
        target_bir_lowering=False,
        debug=False,
        num_devices=N_CORES,
    )
    with tile.TileContext(nc) as tc:
        _kernel_body(tc)
    nc.compile()
    return nc


def kernel(x, W, b, S):
    global LAST_RESULTS
    nc = _CACHE.get("nc")
    if nc is None:
        nc = _build()
        _CACHE["nc"] = nc

    xf = np.asarray(x, np.float32).reshape(ROWS_TOTAL, N)
    SW = (np.asarray(S, np.float32) * np.asarray(W, np.float32))
    wh = np.zeros((NH, WPAD), NP_BF16)
    wh[:, 0:HALF] = SW[ROWS0, 0:HALF]
    wh[:, HALF:N] = SW[ROWS1, HALF:N]
    bfv = np.asarray(b, np.float32).reshape(N)
    bf = np.zeros((2 * NH, BPAD), np.float32)
    bf[0:HALF, 0] = bfv[0:HALF]
    bf[NH : NH + HALF, 0] = bfv[HALF:N]

    in_maps = []
    for i in range(N_CORES):
        xt = xf[i * SHARD : (i + 1) * SHARD].T          # [208, SHARD] view
        xh = np.empty((NR, SHARD), NP_BF16)
        xh[0:K] = xt[N - K : N]
        xh[K : N + K] = xt
        xh[N + K : NR] = xt[0:K]
        in_maps.append({"xh": xh, "wh": wh, "bias": bf})
    res = run_bass_kernel_spmd(nc, in_maps, core_ids=list(range(N_CORES)))
    LAST_RESULTS = res
    out = np.empty((ROWS_TOTAL, N), np.float32)
    for i, r in enumerate(res.results):
        yt = r["outt"]                                  # [224, SHARD] bf16
        out[i * SHARD : (i + 1) * SHARD, 0:HALF] = yt[0:HALF].T
        out[i * SHARD : (i + 1) * SHARD, HALF:N] = yt[NH : NH + HALF].T
    return out.reshape(B, T, N)



# revision 4
# speedup vs baseline: 1.0560x; 1.0560x over previous
"""Locally-connected graph-conv kernel for Trainium2 (Bass/Tile).

Computes out[b,t,m] = sum_n x[b,t,n] * (S*W)[n,m] + bias[m] for
x [64, 2048, 208], W/S [208, 208], bias [208].

The ring-graph support S is a +-4 band (mod 208), so each half of the
output nodes only needs a 112-row slice of the contraction dim. With a
rotated node layout (row j holds node (j-4) mod 208, 216 rows total):
  block 0 (m 0..103):   rotated rows   0..111
  block 1 (m 104..207): rotated rows 104..215
Each output block is a SINGLE [112,104] x [112,512] matmul with the
host-premasked weight block stationary in the PE array and x^T streaming
as the moving operand.

Everything that touches HBM is bf16 (PSUM accumulation stays fp32):
the 2e-2 rel-err budget dwarfs bf16 rounding (~5e-3), and it halves DMA
bytes vs fp32. ~14.9 MB/core of traffic floors the kernel around
~40 us; v2 is shaped to keep the DMA pipe full from the first useful
microsecond:
 - ALL loads ride the Sync ring in strict priority order (wh first,
   then x chunks front-to-back). One ring = one FIFO: the head transfer
   gets the full HBM rate, so chunk 0 completes ~1 us after wh instead
   of round-robining against 5 MB of later chunks (v1 measured the
   first matmul waiting until 16.3 us because wh crawled at 39 GB/s
   behind the x flood split across two rings).
 - PE warm-up no longer depends on any DMA: a GpSimd memset fills a
   dummy tile at ~6.3 us (right after the framework preamble) and 8
   back-to-back dummy matmuls on it burn the ~3.4 us HAM window, so
   HAM un-throttles (1.2 -> 2.4 GHz) right as the first real chunk
   lands. Real matmuls then run at ~2x the v1 rate.
 - chunk sizes taper at BOTH ends ([1k,1k,2k...2k,1k,1k]): small first
   chunks start compute early; small last chunks keep the store tail
   short.
 - stores ride the Scalar ring exclusively (separate FIFO from loads;
   the SDMA engines round-robin the two rings, which is how reads and
   writes share HBM bandwidth). Block-0 eviction on VectorE, block-1 on
   ScalarE (PSUM->SBUF is stuck at 1 elem/lane/cycle fp32-source, one
   engine alone cannot keep up with the warm PE).
 - weights/bias DRAM rows are padded to >=1 KB so their one-time loads
   are not tiny-descriptor crawls; 112-row output tiles keep the
   partition count a multiple of 16 (fast HWDGE path).
The host transposes y^T back at gather.
"""

import numpy as np
import ml_dtypes
from contextlib import ExitStack

import concourse.bacc as bacc
import concourse.mybir as mybir
import concourse.tile as tile
from concourse.bass_utils import run_bass_kernel_spmd

N = 208                      # nodes
HALF = 104                   # output nodes per block
K = 4                        # band half-width of S
NH = 2 * K + HALF            # 112 contraction rows per block (halo incl.)
NR = N + 2 * K               # 216 rotated rows
WPAD = 1024                  # wh DRAM row padding (2 KB rows -> fast DMA)
BPAD = 256                   # bias DRAM row padding (1 KB f32 rows)
N_CORES = 8
B, T = 64, 2048
ROWS_TOTAL = B * T           # 131072
SHARD = ROWS_TOTAL // N_CORES    # 16384 rows per core
TB = 512                     # moving-block columns per matmul (fp32 PSUM max)
TB2 = 2 * TB                 # eviction group (2 PSUM banks)
CHUNKS = [1024, 1024, 2048, 2048, 2048, 2048, 2048, 1024, 1024, 1024, 1024]
assert sum(CHUNKS) == SHARD
N_DUMMY = 8                  # PE warm-up matmuls on the memset tile

FP32 = mybir.dt.float32
BF16 = mybir.dt.bfloat16
NP_BF16 = ml_dtypes.bfloat16
IDENT = mybir.ActivationFunctionType.Identity

# halo row order (indices into the [208] node dim) for each block
ROWS0 = list(range(N - K, N)) + list(range(0, HALF + K))          # 112
ROWS1 = list(range(HALF - K, N)) + list(range(0, K))              # 112

_CACHE = {}
LAST_RESULTS = None          # BassKernelResults of the most recent run


def _kernel_body(tc):
    nc = tc.nc
    # rotated x: row j = node (j-4) mod 208; block0 = rows 0:112,
    # block1 = rows 104:216
    x_d = nc.dram_tensor("xh", [NR, SHARD], BF16, kind="ExternalInput").ap()
    w_d = nc.dram_tensor("wh", [NH, WPAD], BF16, kind="ExternalInput").ap()
    b_d = nc.dram_tensor("bias", [NH, BPAD], FP32, kind="ExternalInput").ap()
    o_d = nc.dram_tensor("outt", [2 * NH, SHARD], BF16, kind="ExternalOutput").ap()

    with ExitStack() as ctx:
        const = ctx.enter_context(tc.tile_pool(name="const", bufs=1))

        # PE warm-up fuel: memset tile, no DMA dependency. GpSimd runs
        # this right after the framework preamble (~6.3 us), so the 8
        # dummy matmuls below start immediately and HAM un-throttles
        # before the first real chunk lands.
        warm = const.tile([128, TB], BF16, tag="warm")
        nc.gpsimd.memset(warm, 1.0)

        # One-time setup: weights lead the Sync ring (first in FIFO =
        # full HBM rate, done in <1 us); bias on GpSimd (off the
        # latency-critical rings).
        wh = const.tile([NH, WPAD], BF16, tag="wh")
        nc.sync.dma_start(wh, w_d)
        bt = const.tile([NH, BPAD], FP32, tag="bt")
        nc.gpsimd.dma_start(bt, b_d)
        bAc = bt[0:HALF, 0:1]
        bBc = bt[0:HALF, 1:2]

        o0p = ctx.enter_context(tc.tile_pool(name="o0p", bufs=3))
        o1p = ctx.enter_context(tc.tile_pool(name="o1p", bufs=3))
        ps0p = ctx.enter_context(tc.tile_pool(name="ps0p", bufs=2, space="PSUM"))
        ps1p = ctx.enter_context(tc.tile_pool(name="ps1p", bufs=2, space="PSUM"))

        # All x loads up-front on the Sync ring into persistent tiles,
        # strictly front-to-back so completion order == consumption
        # order and the head chunk is never starved.
        xts = []
        col = 0
        for c, csz in enumerate(CHUNKS):
            lsl = slice(col, col + csz)
            xh0 = const.tile([NH, csz], BF16, tag=f"xh0_{c}")
            xh1 = const.tile([NH, csz], BF16, tag=f"xh1_{c}")
            nc.sync.dma_start(xh0, x_d[0:NH, lsl])
            nc.sync.dma_start(xh1, x_d[HALF:NR, lsl])
            xts.append((xh0, xh1, col, csz))
            col += csz

        # PE warm-up: HAM un-throttles (1.2 -> 2.4 GHz) after ~3.4us of
        # sustained busy; burn the preamble-to-first-chunk gap on the
        # memset tile (8 x ~430 ns = ~3.4 us). Dummies cycle through the
        # ps0p pool (shape/tag-matched) so no extra PSUM bank is needed.
        for _ in range(N_DUMMY):
            psd = ps0p.tile([HALF, TB2], FP32, tag="ps0")
            nc.tensor.matmul(psd[:, 0:TB], warm[0:NH, 0:HALF],
                             warm[0:NH, 0:TB], start=True, stop=True)

        for c, (xh0, xh1, col, csz) in enumerate(xts):
            tsl = slice(col, col + csz)
            o0_t = o0p.tile([NH, csz], BF16, tag="o0")
            o1_t = o1p.tile([NH, csz], BF16, tag="o1")
            for s in range((csz + TB2 - 1) // TB2):
                g0 = s * TB2
                gw = min(TB2, csz - g0)
                g = slice(g0, g0 + gw)
                # [104, 1024] PSUM tiles (2 banks); each matmul fills one
                ps0 = ps0p.tile([HALF, TB2], FP32, tag="ps0")
                ps1 = ps1p.tile([HALF, TB2], FP32, tag="ps1")
                for q0 in range(0, gw, TB):
                    qs = slice(g0 + q0, g0 + q0 + TB)
                    nc.tensor.matmul(ps0[:, q0 : q0 + TB], wh[:, 0:HALF],
                                     xh0[:, qs], start=True, stop=True)
                    nc.tensor.matmul(ps1[:, q0 : q0 + TB], wh[:, HALF:N],
                                     xh1[:, qs], start=True, stop=True)
                # evictions split across engines: block0 on VectorE,
                # block1 on ScalarE; both fuse the bias and fp32->bf16
                nc.vector.tensor_scalar_add(o0_t[0:HALF, g], ps0[:, 0:gw], bAc)
                nc.scalar.activation(o1_t[0:HALF, g], ps1[:, 0:gw], IDENT, bias=bBc)
            # stores ride the Scalar ring (own FIFO; SDMA round-robins
            # it against the Sync load ring). 112-row DMAs keep the
            # partition count a multiple of 16 (fast HWDGE path).
            nc.scalar.dma_start(o_d[0:NH, tsl], o0_t)
            nc.scalar.dma_start(o_d[NH : 2 * NH, tsl], o1_t)


def _build():
    nc = bacc.Bacc(
        "TRN2",
        target_bir_lowering=False,
        debug=False,
        num_devices=N_CORES,
    )
    with tile.TileContext(nc) as tc:
        _kernel_body(tc)
    nc.compile()
    return nc


def kernel(x, W, b, S):
    global LAST_RESULTS
    nc = _CACHE.get("nc")
    if nc is None:
        nc = _build()
        _CACHE["nc"] = nc

    xf = np.asarray(x, np.float32).reshape(ROWS_TOTAL, N)
    SW = (np.asarray(S, np.float32) * np.asarray(W, np.float32))
    wh = np.zeros((NH, WPAD), NP_BF16)
    wh[:, 0:HALF] = SW[ROWS0, 0:HALF]
    wh[:, HALF:N] = SW[ROWS1, HALF:N]
    bfv = np.asarray(b, np.float32).reshape(N)
    bf = np.zeros((NH, BPAD), np.float32)
    bf[0:HALF, 0] = bfv[0:HALF]
    bf[0:HALF, 1] = bfv[HALF:N]

    in_maps = []
    for i in range(N_CORES):
        xt = xf[i * SHARD : (i + 1) * SHARD].T          # [208, SHARD] view
        xh = np.empty((NR, SHARD), NP_BF16)
        xh[0:K] = xt[N - K : N]
        xh[K : N + K] = xt
        xh[N + K : NR] = xt[0:K]
        in_maps.append({"xh": xh, "wh": wh, "bias": bf})
    res = run_bass_kernel_spmd(nc, in_maps, core_ids=list(range(N_CORES)))
    LAST_RESULTS = res
    out = np.empty((ROWS_TOTAL, N), np.float32)
    for i, r in enumerate(res.results):
        yt = r["outt"]                                  # [224, SHARD] bf16
        out[i * SHARD : (i + 1) * SHARD, 0:HALF] = yt[0:HALF].T
        out[i * SHARD : (i + 1) * SHARD, HALF:N] = yt[NH : NH + HALF].T
    return out.reshape(B, T, N)


# revision 5
# speedup vs baseline: 1.0947x; 1.0367x over previous
"""Locally-connected graph-conv kernel for Trainium2 (Bass/Tile).

Computes out[b,t,m] = sum_n x[b,t,n] * (S*W)[n,m] + bias[m] for
x [64, 2048, 208], W/S [208, 208], bias [208].

The ring-graph support S is a +-4 band (mod 208), so each half of the
output nodes only needs a 112-row slice of the contraction dim. With a
rotated node layout (row j holds node (j-4) mod 208, 216 rows total):
  block 0 (m 0..103):   rotated rows   0..111
  block 1 (m 104..207): rotated rows 104..215
Each output block is a SINGLE [112,104] x [112,512] matmul with the
host-premasked weight block stationary in the PE array and x^T streaming
as the moving operand.

Everything that touches HBM is bf16 (PSUM accumulation stays fp32).
~14.9 MB/core of traffic floors the kernel around ~40 us. Measured HW
behavior this build is tuned against:
 - ONE HWDGE ring sustains only ~260 GB/s; the ~358 GB/s HBM rate needs
   both rings pulling. So loads are split stream-wise: xh0 chunks on
   the Sync ring (wh at its head), xh1 chunks on the Scalar ring (bias
   at its head), strictly front-to-back so the head chunk is never
   starved behind later bytes. Stores go on the GpSimd SWDGE ring - a
   third descriptor stream that round-robins against both load rings
   for write bandwidth without ever queueing a store behind unrelated
   load bytes (and without eating ScalarE issue time, which ScalarE
   needs for evictions).
 - The PE runs at ~1.0 GHz until HAM sees ~3.4 us of sustained busy,
   then ~2.0 GHz (P0) - and HAM re-throttles after windows with
   repeated micro-idle (measured: 12 us cold stretches mid-run from
   ~1 us chunk-boundary gaps). So: 5 dummy matmuls on a memset tile
   (no DMA dependency, start ~7 us) ramp HAM before the first chunk
   lands, and one keep-alive dummy matmul after every chunk's real
   matmuls keeps the duty cycle high enough to hold K=8/8.
 - ScalarE interleaves its per-chunk eviction with the xh1 load issue
   for chunk c+3: load issue stays 3 chunks ahead of consumption, and
   the ring never sits behind an 8 us up-front issue block.
 - chunk sizes taper at BOTH ends: small first chunks start compute
   early, small last chunks keep the store tail short.
 - PSUM->SBUF eviction is 1 elem/lane/cycle (fp32 source), so block 0
   evicts on VectorE and block 1 on ScalarE, both fusing bias and the
   fp32->bf16 down-convert.
The host transposes y^T back at gather.
"""

import numpy as np
import ml_dtypes
from contextlib import ExitStack

import concourse.bacc as bacc
import concourse.mybir as mybir
import concourse.tile as tile
from concourse.bass_utils import run_bass_kernel_spmd

N = 208                      # nodes
HALF = 104                   # output nodes per block
K = 4                        # band half-width of S
NH = 2 * K + HALF            # 112 contraction rows per block (halo incl.)
NR = N + 2 * K               # 216 rotated rows
WPAD = 1024                  # wh DRAM row padding (2 KB rows -> fast DMA)
BPAD = 256                   # bias DRAM row padding (1 KB f32 rows)
N_CORES = 8
B, T = 64, 2048
ROWS_TOTAL = B * T           # 131072
SHARD = ROWS_TOTAL // N_CORES    # 16384 rows per core
TB = 512                     # moving-block columns per matmul (fp32 PSUM max)
TB2 = 2 * TB                 # eviction group (2 PSUM banks)
CHUNKS = [1024, 1024, 2048, 2048, 2048, 2048, 2048, 1024, 1024, 1024, 1024]
assert sum(CHUNKS) == SHARD
N_DUMMY = 5                  # PE warm-up matmuls on the memset tile
PREF = 3                     # chunks of xh1-load-issue lookahead on ScalarE

FP32 = mybir.dt.float32
BF16 = mybir.dt.bfloat16
NP_BF16 = ml_dtypes.bfloat16
IDENT = mybir.ActivationFunctionType.Identity

# halo row order (indices into the [208] node dim) for each block
ROWS0 = list(range(N - K, N)) + list(range(0, HALF + K))          # 112
ROWS1 = list(range(HALF - K, N)) + list(range(0, K))              # 112

_CACHE = {}
LAST_RESULTS = None          # BassKernelResults of the most recent run


def _kernel_body(tc):
    nc = tc.nc
    # rotated x: row j = node (j-4) mod 208; block0 = rows 0:112,
    # block1 = rows 104:216
    x_d = nc.dram_tensor("xh", [NR, SHARD], BF16, kind="ExternalInput").ap()
    w_d = nc.dram_tensor("wh", [NH, WPAD], BF16, kind="ExternalInput").ap()
    b_d = nc.dram_tensor("bias", [NH, BPAD], FP32, kind="ExternalInput").ap()
    o_d = nc.dram_tensor("outt", [2 * NH, SHARD], BF16, kind="ExternalOutput").ap()

    with ExitStack() as ctx:
        const = ctx.enter_context(tc.tile_pool(name="const", bufs=1))

        # PE warm-up fuel: memset tile, no DMA dependency, ready right
        # after the framework preamble.
        warm = const.tile([NH, TB], BF16, tag="warm")
        nc.gpsimd.memset(warm, 1.0)

        # Ring heads: wh leads Sync, bias leads Scalar (both tiny, done
        # in <1 us at the head of their FIFOs).
        wh = const.tile([NH, WPAD], BF16, tag="wh")
        nc.sync.dma_start(wh, w_d)
        bt = const.tile([NH, BPAD], FP32, tag="bt")
        nc.scalar.dma_start(bt, b_d)
        bAc = bt[0:HALF, 0:1]
        bBc = bt[0:HALF, 1:2]

        o0p = ctx.enter_context(tc.tile_pool(name="o0p", bufs=3))
        o1p = ctx.enter_context(tc.tile_pool(name="o1p", bufs=3))
        ps0p = ctx.enter_context(tc.tile_pool(name="ps0p", bufs=2, space="PSUM"))
        ps1p = ctx.enter_context(tc.tile_pool(name="ps1p", bufs=2, space="PSUM"))

        # persistent x tiles; loads issued chunk-order, xh0 on Sync.
        # xh1 on Scalar, interleaved with evictions below (PREF ahead).
        xts = []
        col = 0
        for c, csz in enumerate(CHUNKS):
            xh0 = const.tile([NH, csz], BF16, tag=f"xh0_{c}")
            xh1 = const.tile([NH, csz], BF16, tag=f"xh1_{c}")
            xts.append((xh0, xh1, col, csz))
            col += csz

        def issue_loads(c):
            xh0, xh1, col, csz = xts[c]
            lsl = slice(col, col + csz)
            nc.sync.dma_start(xh0, x_d[0:NH, lsl])
            nc.scalar.dma_start(xh1, x_d[HALF:NR, lsl])

        for c in range(PREF):
            issue_loads(c)

        # PE warm-up: HAM un-throttles (~1.0 -> ~2.0 GHz) after ~3.4us
        # of sustained busy; burn the preamble-to-first-chunk gap on the
        # memset tile (5 x ~740 ns cold = ~3.7 us). Dummies cycle
        # through ps0p (shape/tag-matched) so no extra PSUM bank.
        def dummy_mm():
            psd = ps0p.tile([HALF, TB2], FP32, tag="ps0")
            nc.tensor.matmul(psd[:, 0:TB], warm[:, 0:HALF], warm,
                             start=True, stop=True)

        for _ in range(N_DUMMY):
            dummy_mm()

        n_chunks = len(CHUNKS)
        for c, (xh0, xh1, col, csz) in enumerate(xts):
            tsl = slice(col, col + csz)
            o0_t = o0p.tile([NH, csz], BF16, tag="o0")
            o1_t = o1p.tile([NH, csz], BF16, tag="o1")
            for s in range((csz + TB2 - 1) // TB2):
                g0 = s * TB2
                gw = min(TB2, csz - g0)
                g = slice(g0, g0 + gw)
                # [104, 1024] PSUM tiles (2 banks); each matmul fills one
                ps0 = ps0p.tile([HALF, TB2], FP32, tag="ps0")
                ps1 = ps1p.tile([HALF, TB2], FP32, tag="ps1")
                for q0 in range(0, gw, TB):
                    qs = slice(g0 + q0, g0 + q0 + TB)
                    nc.tensor.matmul(ps0[:, q0 : q0 + TB], wh[:, 0:HALF],
                                     xh0[:, qs], start=True, stop=True)
                    nc.tensor.matmul(ps1[:, q0 : q0 + TB], wh[:, HALF:N],
                                     xh1[:, qs], start=True, stop=True)
                # evictions split across engines: block0 on VectorE,
                # block1 on ScalarE; both fuse the bias and fp32->bf16
                nc.vector.tensor_scalar_add(o0_t[0:HALF, g], ps0[:, 0:gw], bAc)
                nc.scalar.activation(o1_t[0:HALF, g], ps1[:, 0:gw], IDENT, bias=bBc)
            # HAM keep-alive: one dummy matmul fills the chunk-boundary
            # PE gap so the activity window never reads idle.
            if c + 1 < n_chunks:
                dummy_mm()
            # next xh1 load issue lands here in ScalarE program order,
            # keeping issue PREF chunks ahead of consumption.
            if c + PREF < n_chunks:
                issue_loads(c + PREF)
            # stores on the GpSimd SWDGE ring: own FIFO, never behind
            # loads. 112-row DMAs (partition count multiple of 16).
            nc.gpsimd.dma_start(o_d[0:NH, tsl], o0_t)
            nc.gpsimd.dma_start(o_d[NH : 2 * NH, tsl], o1_t)


def _build():
    nc = bacc.Bacc(
        "TRN2",
        target_bir_lowering=False,
        debug=False,
        num_devices=N_CORES,
    )
    with tile.TileContext(nc) as tc:
        _kernel_body(tc)
    nc.compile()
    return nc


def kernel(x, W, b, S):
    global LAST_RESULTS
    nc = _CACHE.get("nc")
    if nc is None:
        nc = _build()
        _CACHE["nc"] = nc

    xf = np.asarray(x, np.float32).reshape(ROWS_TOTAL, N)
    SW = (np.asarray(S, np.float32) * np.asarray(W, np.float32))
    wh = np.zeros((NH, WPAD), NP_BF16)
    wh[:, 0:HALF] = SW[ROWS0, 0:HALF]
    wh[:, HALF:N] = SW[ROWS1, HALF:N]
    bfv = np.asarray(b, np.float32).reshape(N)
    bf = np.zeros((NH, BPAD), np.float32)
    bf[0:HALF, 0] = bfv[0:HALF]
    bf[0:HALF, 1] = bfv[HALF:N]

    in_maps = []
    for i in range(N_CORES):
        xt = xf[i * SHARD : (i + 1) * SHARD].T          # [208, SHARD] view
        xh = np.empty((NR, SHARD), NP_BF16)
        xh[0:K] = xt[N - K : N]
        xh[K : N + K] = xt
        xh[N + K : NR] = xt[0:K]
        in_maps.append({"xh": xh, "wh": wh, "bias": bf})
    res = run_bass_kernel_spmd(nc, in_maps, core_ids=list(range(N_CORES)))
    LAST_RESULTS = res
    out = np.empty((ROWS_TOTAL, N), np.float32)
    for i, r in enumerate(res.results):
        yt = r["outt"]                                  # [224, SHARD] bf16
        out[i * SHARD : (i + 1) * SHARD, 0:HALF] = yt[0:HALF].T
        out[i * SHARD : (i + 1) * SHARD, HALF:N] = yt[NH : NH + HALF].T
    return out.reshape(B, T, N)
